# revision 18
# baseline (speedup 1.0000x reference)
"""Ex2Vec Trainium2 Bass kernel (v2).

Data-parallel over batch B=32 across 8 NeuronCores (4 batch rows/core).
The augmented item table is replicated; gathers run on-device via
indirect DMA.

Math per batch row b (reference):
    s[i,j] = |h_i - p_j|^2,  d = sqrt(max(s,0)+eps)
    kern   = sigmoid(smooth/(1+d) - force*smooth) / denom
    res_j  = lamb * sum_i td_i kern[i,j],   td = (t+cutoff)^-.5 * w
    out    = relu(sqrt(|u-p|^2+eps) - res)
    I      = alpha*out + beta*out^2 + gamma + user_bias + item_bias

Device strategy:
  - kern(s) is approximated by A + f*s + B*exp(u*s) (host-fitted per
    scalar set, pinned at s=0; max err ~5e-3 on the data's s-range).
    The exp() is ONE scalar-engine pass per batch row reading the Gram
    PSUM; the A and f*s terms of the td-reduction expand analytically:
        sum_i td_i s_ij = K + T*pp_j - 2*(sum_i td_i h_i) . p_j
    so they ride the same indicator-masked accumulation matmul that
    computes |u-p|^2 (extra lhsT columns) -- no elementwise passes.
  - every transcendental comes from ONE ACT table set
    (natural_log_exp_and_others): sqrt(x)=exp(0.5*ln x),
    (t+c)^-0.5 = exp(-0.5*ln(t+c)), kern uses exp. One table load.
  - all big matmuls run f32r (4x fp32 rate at >=256 free).
  - gathers: one batched indirect DMA per batch row ([1,1152] offsets,
    ~1us fixed cost per call) when GATHER_BATCHED, else 9x [128,1].
"""

import os
import numpy as np
from contextlib import ExitStack

import concourse.bass as bass
import concourse.bacc as bacc
import concourse.mybir as mybir
import concourse.tile as tile
from concourse.masks import make_identity
from concourse.bass_utils import run_bass_kernel_spmd

F32 = mybir.dt.float32
F32R = mybir.dt.float32r
I32 = mybir.dt.int32
AF = mybir.ActivationFunctionType
OP = mybir.AluOpType
AX = mybir.AxisListType

NCORES = 8
B = 32
BPC = B // NCORES          # 4 batch rows per core
P_REAL = 1000
PP = 1024                  # padded pred count
NCH = PP // 128            # 8 chunks of 128 pred rows
NI = NCH + 1               # chunks + history column
H = 128
D = 64
V = 100001
W = D + 1                  # gathered row: emb | item_bias
EPSP = 4e-3                # eps' folded into the Gram (keeps sqrt-args > 0)

GATHER_BATCHED = bool(int(os.environ.get("K_GBATCH", "0")))
# batched-gather index ordering: "pj" -> k = p*NI + j ; "jp" -> k = j*128 + p
GATHER_ORDER = os.environ.get("K_GORDER", "pj")

_cache: dict = {}


def _sigmoid(x):
    return 1.0 / (1.0 + np.exp(-x))


def _fit_kern(smooth, force, smax=3.4, n=1701):
    """Fit kern(s) ~= A + f*s + B*exp(u*s) on s in [0, smax], pinned at 0.

    s is the DEVICE Gram value (true |h-p|^2 + EPSP); returns (A, f, u, v)
    with v = ln(B) so the device computes exp(u*s + v).
    """
    denom = _sigmoid(smooth - force * smooth)
    s = np.linspace(0.0, smax, n)
    st = np.maximum(s - EPSP, 0.0)
    tgt = _sigmoid(smooth / (1.0 + np.sqrt(st)) - force * smooth) / denom
    wgt = np.ones_like(s)
    wgt[0] = 500.0
    best = None
    for u in np.linspace(-8.0, -0.6, 149):
        X = np.stack([np.ones_like(s), s, np.exp(u * s)], 1)
        c, *_ = np.linalg.lstsq(X * wgt[:, None], tgt * wgt, rcond=None)
        if c[2] <= 1e-8:
            continue
        err = np.abs(X @ c - tgt).max()
        if best is None or err < best[0]:
            best = (err, u, c)
    assert best is not None
    _, u, (A, f, Bc) = best
    return float(A), float(f), float(u), float(np.log(Bc))


def _build(scalars):
    (global_lamb, alpha, beta, gamma, cutoff, smooth, force) = scalars
    A, f, u, v = _fit_kern(smooth, force)

    nc = bacc.Bacc("TRN2", target_bir_lowering=False, debug=False,
                   num_devices=NCORES)

    # activation-bias constants must exist as const APs before use
    for cv in sorted({0.0, float(cutoff), float(v)}):
        if (F32, cv) not in nc.const_aps.aps:
            t = nc.alloc_sbuf_tensor(f"constap-{cv}", [128, 1], F32)
            nc.gpsimd.memset(t.ap(), cv)
            nc.const_aps.aps[(F32, cv)] = t.ap()
    nc.all_engine_barrier()

    # ---- DRAM I/O ------------------------------------------------------
    t_aug = nc.dram_tensor("aug_item", [V, W], F32, kind="ExternalInput")
    t_uaug = nc.dram_tensor("user_aug", [V, D + 2], F32, kind="ExternalInput")
    if GATHER_BATCHED:
        t_idx = nc.dram_tensor("idx_ph", [BPC, 128 * NI], I32,
                               kind="ExternalInput")
    else:
        t_idx = nc.dram_tensor("idx_ph", [128, BPC * NI], I32,
                               kind="ExternalInput")
    t_idxu = nc.dram_tensor("idx_user", [BPC, 1], I32, kind="ExternalInput")
    t_td = nc.dram_tensor("tdelta", [BPC, H], F32, kind="ExternalInput")
    t_wt = nc.dram_tensor("tweight", [BPC, H], F32, kind="ExternalInput")
    t_out = nc.dram_tensor("out", [BPC, PP], F32, kind="ExternalOutput")

    with tile.TileContext(nc) as tc, ExitStack() as ctx:
        const = ctx.enter_context(tc.tile_pool(name="const", bufs=1))
        sb1 = ctx.enter_context(tc.tile_pool(name="sb1", bufs=1))

        ps_s = ctx.enter_context(tc.tile_pool(name="ps_s", bufs=2,
                                              space="PSUM"))
        ps_pT = ctx.enter_context(tc.tile_pool(name="ps_pT", bufs=1,
                                               space="PSUM"))
        ps_acc = ctx.enter_context(tc.tile_pool(name="ps_acc", bufs=1,
                                                space="PSUM"))
        ps_misc = ctx.enter_context(tc.tile_pool(name="ps_misc", bufs=1,
                                                 space="PSUM"))

        ident = const.tile([128, 128], F32)
        make_identity(nc, ident[:])

        # ---- input loads ----------------------------------------------
        if GATHER_BATCHED:
            idx_sb = sb1.tile([BPC, 128 * NI], I32)
        else:
            idx_sb = sb1.tile([128, BPC * NI], I32)
        idxu = sb1.tile([BPC, 1], I32)
        td_in = sb1.tile([BPC, H], F32)
        wt_in = sb1.tile([BPC, H], F32)
        nc.sync.dma_start(out=idx_sb[:], in_=t_idx[:])
        nc.sync.dma_start(out=idxu[:], in_=t_idxu[:])
        nc.sync.dma_start(out=td_in[:], in_=t_td[:])
        nc.sync.dma_start(out=wt_in[:], in_=t_wt[:])

        # ---- user gather + gathers (gpsimd queue, emitted first) ------
        uaug = sb1.tile([BPC, D + 2], F32)
        nc.gpsimd.indirect_dma_start(
            out=uaug[:], out_offset=None, in_=t_uaug[:],
            in_offset=bass.IndirectOffsetOnAxis(ap=idxu[:, :1], axis=0))

        p_all_tiles = []
        for b in range(BPC):
            p_all = sb1.tile([128, NI, 68], F32, name=f"p_all{b}")
            p_all_tiles.append(p_all)
            if GATHER_BATCHED:
                nc.gpsimd.indirect_dma_start(
                    out=p_all[:, :, 2:2 + W], out_offset=None, in_=t_aug[:],
                    in_offset=bass.IndirectOffsetOnAxis(
                        ap=idx_sb[b:b + 1, :], axis=0))
            else:
                for c in range(NI):
                    nc.gpsimd.indirect_dma_start(
                        out=p_all[:, c, 2:2 + W], out_offset=None,
                        in_=t_aug[:],
                        in_offset=bass.IndirectOffsetOnAxis(
                            ap=idx_sb[:, b * NI + c:b * NI + c + 1], axis=0))

        # ---- user/td prep ---------------------------------------------
        ul = uaug[:, D:D + 1]
        ub = uaug[:, D + 1:D + 2]

        usq = sb1.tile([BPC, D], F32)
        uu = sb1.tile([BPC, 1], F32)
        nc.vector.tensor_mul(usq[:], uaug[:, 0:D], uaug[:, 0:D])
        nc.vector.reduce_sum(uu[:], usq[:], axis=AX.X)

        # urows[b] = [1, uu_b+eps', -2*u_b]   (squared-u-dist lhsT column)
        urows = sb1.tile([BPC, 66], F32)
        nc.vector.memset(urows[:, 0:1], 1.0)
        nc.vector.tensor_scalar_add(urows[:, 1:2], uu[:], EPSP)
        nc.vector.tensor_scalar_mul(urows[:, 2:66], uaug[:, 0:D], -2.0)

        # td' = (t+cutoff)^-0.5 * w * lamb   via exp(-0.5*ln(t+cutoff))
        tds = sb1.tile([BPC, H], F32)
        tdr = sb1.tile([BPC, H], F32)
        tdl = sb1.tile([BPC, H], F32)
        lamb_c = sb1.tile([BPC, 1], F32)
        nc.scalar.activation(tds[:], td_in[:], AF.Ln, bias=float(cutoff))
        nc.scalar.activation(tdr[:], tds[:], AF.Exp, scale=-0.5)
        nc.vector.tensor_scalar_add(lamb_c[:], ul, float(global_lamb))
        nc.vector.scalar_tensor_tensor(
            out=tdl[:], in0=tdr[:], scalar=lamb_c[:, :1], in1=wt_in[:],
            op0=OP.mult, op1=OP.mult)
        T_b = sb1.tile([BPC, 1], F32)
        AT_b = sb1.tile([BPC, 1], F32)
        nc.vector.reduce_sum(T_b[:], tdl[:], axis=AX.X)
        nc.vector.tensor_scalar_mul(AT_b[:], T_b[:], float(A))

        # td4m [128, 4*BPC]: slot b holds td'_b at column b (else 0)
        ps_td = ps_misc.tile([128, BPC], F32, space="PSUM", tag="m")
        nc.tensor.transpose(ps_td[:, 0:BPC], tdl[:], ident[0:BPC, 0:BPC])
        td4m = sb1.tile([128, BPC * BPC], F32R)
        nc.vector.memset(td4m[:].bitcast(F32), 0.0)
        for b in range(BPC):
            nc.vector.tensor_copy(td4m[:, b * BPC + b:b * BPC + b + 1],
                                  ps_td[:, b:b + 1])
        # T and A*T as partition-0 rows [1, BPC] (for per-b s-col builds)
        ps_T = ps_misc.tile([1, 2 * BPC], F32, space="PSUM", tag="m")
        nc.tensor.transpose(ps_T[0:1, 0:BPC], T_b[:], ident[0:BPC, 0:BPC])
        nc.tensor.transpose(ps_T[0:1, BPC:2 * BPC], AT_b[:],
                            ident[0:BPC, 0:BPC])
        Trow = sb1.tile([1, 2 * BPC], F32)
        nc.vector.tensor_copy(Trow[:], ps_T[0:1, :])

        # u4m [66, 8*BPC]: slot b: col b = u-col, col 4+b = f*s-col
        ps_uT = ps_misc.tile([66, BPC], F32, space="PSUM", tag="m")
        nc.tensor.transpose(ps_uT[0:66, 0:BPC], urows[:], ident[0:BPC, 0:BPC])
        u4m = sb1.tile([66, 8 * BPC], F32R)
        nc.vector.memset(u4m[:].bitcast(F32), 0.0)
        for b in range(BPC):
            nc.vector.tensor_copy(u4m[:, 8 * b + b:8 * b + b + 1],
                                  ps_uT[:, b:b + 1])

        # gamma + user_bias broadcast to all partitions: ubg_b [128, BPC]
        ones_row = sb1.tile([1, 128], F32)
        nc.vector.memset(ones_row[:], 1.0)
        ubg = sb1.tile([BPC, 1], F32)
        nc.vector.tensor_scalar_add(ubg[:], ub, float(gamma))
        ps_ubt = ps_pT.tile([128, 512], F32, space="PSUM", tag="pT")
        nc.tensor.transpose(ps_ubt[0:1, 0:BPC], ubg[:], ident[0:BPC, 0:BPC])
        ubg_row = sb1.tile([1, BPC], F32)
        nc.vector.tensor_copy(ubg_row[:], ps_ubt[0:1, 0:BPC])
        ps_ubb = ps_pT.tile([128, 512], F32, space="PSUM", tag="pT")
        nc.tensor.matmul(ps_ubb[:, 0:BPC], lhsT=ones_row[:], rhs=ubg_row[:],
                         start=True, stop=True)
        ubg_b = sb1.tile([128, BPC], F32)
        nc.vector.tensor_copy(ubg_b[:], ps_ubb[:, 0:BPC])

        # ---- main loop -------------------------------------------------
        # acc_e rows 0:4: res (exp part). acc_u rows 0:4 squ, 4:8 f*s-red
        acc_e = [ps_acc.tile([BPC, 512], F32, space="PSUM", tag=f"acce{g}",
                             name=f"acce{g}") for g in range(2)]
        acc_u = [ps_acc.tile([8, 512], F32, space="PSUM", tag=f"accu{g}",
                             name=f"accu{g}") for g in range(2)]
        e_tiles = []
        p_sides = [[None, None] for _ in range(BPC)]
        for b in range(BPC):
            p_all = p_all_tiles[b]
            nc.vector.memset(p_all[:, 0:NCH, 1:2], 1.0)   # ones col
            # |h|^2 and h_aug = [1, hh+eps', -2h]
            hsq = sb1.tile([128, D], F32, tag="hsq")
            hh = sb1.tile([128, 1], F32, tag="hh")
            h_aug = sb1.tile([128, 66], F32, tag="h_aug")
            nc.vector.tensor_mul(hsq[:], p_all[:, NCH, 2:2 + D],
                                 p_all[:, NCH, 2:2 + D])
            nc.vector.reduce_sum(hh[:], hsq[:], axis=AX.X)
            nc.vector.memset(h_aug[:, 0:1], 1.0)
            nc.vector.tensor_scalar_add(h_aug[:, 1:2], hh[:], EPSP)
            nc.vector.tensor_scalar_mul(h_aug[:, 2:66],
                                        p_all[:, NCH, 2:2 + D], -2.0)

            # |p|^2 directly into p_all col 0
            psq = sb1.tile([128, NCH * D], F32, tag="psq")
            nc.vector.tensor_mul(psq[:], p_all[:, 0:NCH, 2:2 + D],
                                 p_all[:, 0:NCH, 2:2 + D])
            nc.vector.reduce_sum(
                p_all[:, 0:NCH, 0:1],
                psq[:].rearrange("p (c d) -> p c d", c=NCH), axis=AX.X)

            # h side transpose
            ps_h = ps_pT.tile([128, 512], F32, space="PSUM", tag="pT")
            nc.tensor.transpose(ps_h[0:66, 0:128], h_aug[:], ident[:])
            h_side = sb1.tile([66, 128], F32R, tag="h_side")
            nc.vector.tensor_copy(h_side[:], ps_h[0:66, 0:128])

            # th row: [sum td'(hh+eps'), -2*sum td' h] at partition 0
            th_b = ps_misc.tile([1, 65], F32, space="PSUM", tag="m")
            nc.tensor.matmul(th_b[0:1, :],
                             lhsT=td4m[:, BPC * b + b:BPC * b + b + 1]
                                 .bitcast(F32),
                             rhs=h_aug[:, 1:66],
                             start=True, stop=True)
            # s-col slot b: f * [T_b, K'_b + (A/f)T_b, -2 th_b]
            srow = sb1.tile([1, 66], F32, tag="srow")
            nc.vector.tensor_scalar_mul(srow[:, 0:1], Trow[:, b:b + 1],
                                        float(f))
            nc.vector.scalar_tensor_tensor(
                out=srow[:, 1:2], in0=th_b[0:1, 0:1], scalar=float(f),
                in1=Trow[:, BPC + b:BPC + b + 1], op0=OP.mult, op1=OP.add)
            nc.vector.tensor_scalar_mul(srow[:, 2:66], th_b[0:1, 1:65],
                                        float(f))
            ps_sc = ps_misc.tile([66, 1], F32, space="PSUM", tag="m")
            nc.tensor.transpose(ps_sc[0:66, 0:1], srow[:], ident[0:1, 0:1])
            nc.vector.tensor_copy(u4m[:, 8 * b + 4 + b:8 * b + 4 + b + 1],
                                  ps_sc[0:66, 0:1])

            e_sb = sb1.tile([128, PP], F32R, name=f"e{b}")
            e_tiles.append(e_sb)
            for g in range(2):
                ps_p = ps_pT.tile([128, 512], F32, space="PSUM", tag="pT")
                for cc in range(4):
                    c = g * 4 + cc
                    nc.tensor.transpose(ps_p[0:67, cc * 128:(cc + 1) * 128],
                                        p_all[:, c, 0:67], ident[:])
                p_side = sb1.tile([66, 512], F32R, tag=f"p_side{b}{g}",
                                  name=f"p_side{b}{g}")
                p_sides[b][g] = p_side
                nc.vector.tensor_copy(p_side[:], ps_p[0:66, :])

                s_ps = ps_s.tile([128, 512], F32, space="PSUM", tag="s")
                nc.tensor.matmul(s_ps[:], lhsT=h_side[:], rhs=p_side[:],
                                 start=True, stop=True)
                # e = exp(u*s + v)
                nc.scalar.activation(e_sb[:, g * 512:(g + 1) * 512], s_ps[:],
                                     AF.Exp, bias=float(v), scale=float(u))
                # res (exp part): rows 0:4 of acc
                nc.tensor.matmul(acc_e[g][:],
                                 lhsT=td4m[:, BPC * b:BPC * (b + 1)],
                                 rhs=e_sb[:, g * 512:(g + 1) * 512],
                                 start=(b == 0), stop=(b == BPC - 1),
                                 skip_group_check=True)
                # squ rows 0:4 and f*s-reduce rows 4:8
                nc.tensor.matmul(acc_u[g][:],
                                 lhsT=u4m[:, 8 * b:8 * (b + 1)],
                                 rhs=p_sides[b][g][:],
                                 start=(b == 0), stop=(b == BPC - 1),
                                 skip_group_check=True)

        # ---- finals ----------------------------------------------------
        rs_e = sb1.tile([BPC, PP], F32)
        rs_u = sb1.tile([8, PP], F32)
        for g in range(2):
            nc.vector.tensor_copy(rs_e[:, g * 512:(g + 1) * 512],
                                  acc_e[g][:])
            nc.vector.tensor_copy(rs_u[:, g * 512:(g + 1) * 512],
                                  acc_u[g][:])
        cols = ps_misc.tile([128, NCH, 12], F32, space="PSUM", tag="m")
        for c in range(NCH):
            nc.tensor.transpose(cols[:, c, 0:BPC],
                                rs_e[:, c * 128:(c + 1) * 128],
                                ident[0:BPC, 0:BPC])
            nc.tensor.transpose(cols[:, c, BPC:12],
                                rs_u[:, c * 128:(c + 1) * 128],
                                ident[0:8, 0:8])

        # dist_ui = exp(0.5*ln(squ));  res = exp-part + f*s-part
        lnq = sb1.tile([128, NCH, BPC], F32)
        dui = sb1.tile([128, NCH, BPC], F32)
        rc = sb1.tile([128, NCH, BPC], F32)
        nc.scalar.activation(lnq[:], cols[:, :, BPC:2 * BPC], AF.Ln)
        nc.scalar.activation(dui[:], lnq[:], AF.Exp, scale=0.5)
        rc1 = sb1.tile([128, NCH, BPC], F32)
        nc.vector.tensor_copy(rc1[:], cols[:, :, 0:BPC])
        nc.vector.tensor_add(rc[:], rc1[:], cols[:, :, 2 * BPC:])

        o = sb1.tile([128, NCH, BPC], F32)
        q2 = sb1.tile([128, NCH, BPC], F32)
        m = sb1.tile([128, NCH, BPC], F32)
        ivc = sb1.tile([128, NCH, BPC], F32)
        nc.vector.tensor_sub(o[:], dui[:], rc[:])
        nc.vector.tensor_scalar_max(o[:], o[:], 0.0)
        nc.vector.tensor_scalar(q2[:], o[:], float(beta), float(alpha),
                                op0=OP.mult, op1=OP.add)
        nc.vector.tensor_mul(m[:], q2[:], o[:])
        for b in range(BPC):
            nc.vector.scalar_tensor_tensor(
                out=ivc[:, :, b:b + 1], in0=m[:, :, b:b + 1],
                scalar=ubg_b[:, b:b + 1],
                in1=p_all_tiles[b][:, 0:NCH, 66:67],
                op0=OP.add, op1=OP.add)

        irows = sb1.tile([BPC, PP], F32)
        for g in range(2):
            fin = ps_s.tile([128, 512], F32, space="PSUM", tag="s")
            for cc in range(4):
                c = g * 4 + cc
                nc.tensor.transpose(fin[0:BPC, cc * 128:(cc + 1) * 128],
                                    ivc[:, c, :], ident[:])
            nc.vector.tensor_copy(irows[:, g * 512:(g + 1) * 512],
                                  fin[0:BPC, :])
        nc.sync.dma_start(out=t_out[:], in_=irows[:])

    nc.compile()
    return nc


def _get_nc(scalars):
    key = tuple(float(s) for s in scalars) + (GATHER_BATCHED, GATHER_ORDER)
    if key not in _cache:
        _cache[key] = _build(tuple(float(s) for s in scalars))
    return _cache[key]


def _make_in_maps(inputs):
    user_index = np.asarray(inputs["user_index"]).astype(np.int32)
    pred = np.asarray(inputs["pred_item_indices"]).astype(np.int32)
    hist = np.asarray(inputs["history_item_indices"]).astype(np.int32)
    tdelta = np.asarray(inputs["history_timedeltas"], dtype=np.float32)
    weights = np.asarray(inputs["history_weights"], dtype=np.float32)
    emb_user = np.asarray(inputs["embedding_user"], dtype=np.float32)
    emb_item = np.asarray(inputs["embedding_item"], dtype=np.float32)
    user_lamb = np.asarray(inputs["user_lamb"], dtype=np.float32)
    user_bias = np.asarray(inputs["user_bias"], dtype=np.float32)
    item_bias = np.asarray(inputs["item_bias"], dtype=np.float32)

    aug_item = np.ascontiguousarray(
        np.concatenate([emb_item, item_bias], axis=1), dtype=np.float32)
    user_aug = np.ascontiguousarray(
        np.concatenate([emb_user, user_lamb, user_bias], axis=1),
        dtype=np.float32)

    pred_pad = np.zeros((B, PP), np.int32)
    pred_pad[:, :P_REAL] = pred

    in_maps = []
    for cid in range(NCORES):
        sl = slice(cid * BPC, (cid + 1) * BPC)
        # desired[p, j]: j<8 -> pred chunk j row p ; j==8 -> history row p
        des = np.empty((BPC, 128, NI), np.int32)
        for bi, b in enumerate(range(cid * BPC, (cid + 1) * BPC)):
            des[bi, :, 0:NCH] = pred_pad[b].reshape(NCH, 128).T
            des[bi, :, NCH] = hist[b]
        if GATHER_BATCHED:
            if GATHER_ORDER == "pj":
                idx = des.reshape(BPC, 128 * NI)
            else:  # "jp": k = j*128 + p
                idx = des.transpose(0, 2, 1).reshape(BPC, NI * 128)
            idx_ph = np.ascontiguousarray(idx)
        else:
            idx_ph = np.empty((128, BPC * NI), np.int32)
            for bi in range(BPC):
                idx_ph[:, bi * NI:(bi + 1) * NI] = des[bi]
            idx_ph = np.ascontiguousarray(idx_ph)
        in_maps.append({
            "aug_item": aug_item,
            "user_aug": user_aug,
            "idx_ph": idx_ph,
            "idx_user": np.ascontiguousarray(user_index[sl, None]),
            "tdelta": tdelta[sl],
            "tweight": weights[sl],
        })
    return in_maps


def kernel(**inputs) -> np.ndarray:
    scalars = tuple(float(np.asarray(inputs[k])) for k in
                    ("global_lamb", "alpha", "beta", "gamma", "cutoff",
                     "smooth", "force"))
    nc = _get_nc(scalars)
    in_maps = _make_in_maps(inputs)

    res = run_bass_kernel_spmd(
        nc, in_maps, core_ids=list(range(NCORES)),
        trace=bool(int(os.environ.get("K_TRACE", "0"))))
    if res.exec_time_ns is not None:
        kernel.last_exec_time_ns = res.exec_time_ns
    kernel.last_results = res

    out = np.concatenate([res.results[c]["out"][:, :P_REAL]
                          for c in range(NCORES)], axis=0)
    return np.ascontiguousarray(out, dtype=np.float32)


if __name__ == "__main__":
    import reference
    inputs = {k: np.asarray(v) for k, v in reference.setup_inputs().items()}
    expected = np.asarray(reference.reference(**reference.setup_inputs()))
    actual = kernel(**inputs)
    err = np.abs(actual - expected)
    rel = err.max() / np.abs(expected).max()
    print("max abs err:", err.max(), "rel:", rel)
